# revision 30
# baseline (speedup 1.0000x reference)
"""Trainium2 Bass kernel for ComputeGsct.

Math (per batch b, reduced over N voxels):
    kai(n)   = 10*x2[n,0] - i * x2[n,1]/(OMEGA*EPS0)          (complex scalar)
    A_n      = kai(n) * Gsr_n                                  (complex 3x3)
    C_b      = sum_n A_n @ Grf_n                               (complex 3x3)
    out[b,m,:] = (Re C_b, Im C_b) flattened row-major.

Strategy (v2, "uv" formulation):
  - Batch-parallel sharding: 8 cores x 4 batches each, full N per core.
    Output is concatenated on host - no cross-core reduction needed.
  - Instead of forming A = kai*Gsr on-device (mul+mul+sub+add), compute
    only the two raw broadcast products u = x2r*Gsr and v = x2i*Gsr
    (constants 10 and -1/(OMEGA*EPS0) are folded into the host fixup,
    which is linear in the PSUM accumulator).  One engine each:
    GPSIMD does u, DVE does v, ACT casts Grf to fp16.
  - Per 4-voxel group, ONE fp16 TensorE matmul with stationary
    Grf[128,72] and moving [u|v][128,144], accumulated into a PSUM
    [72,144] tile across all of N.  The 4 diagonal [18,18] blocks of
    each 72-wide half hold sum_n Grf x u and sum_n Grf x v; the host
    fixup extracts the 9 complex entries of C_b.
  - HW-time budget per core (target_regime=memory): ~80 MB of input at
    the measured ~425 GB/s per-core DMA rate => ~187 us floor; every
    compute engine is under ~3 us per 5.9 us tile.
"""

import sys

import numpy as np

_TRN_REPO = "/opt/trn_rl_repo"
if _TRN_REPO not in sys.path:
    sys.path.insert(0, _TRN_REPO)

_PAI = 3.141592653589793
_C = 299792458.0
_OMEGA = 2.0 * _PAI * 2.4e9
_MU0 = 4.0 * _PAI * 1e-7
_EPSILON0 = 1.0 / (_C**2 * _MU0)
_KI_SCALE = -1.0 / (_OMEGA * _EPSILON0)

B_FULL, N_FULL = 32, 131072
N_CORES = 8
B_PC = B_FULL // N_CORES  # batches per core
P = 128  # SBUF partitions == matmul contraction size
KGRP = 4  # voxel-chunks fused per matmul (diag-block trick)
FD_S = 18 * KGRP  # stationary free dim (Grf cols)
FD_M = 2 * 18 * KGRP  # moving free dim ([u|v] cols)


def build_nc(b_pc=B_PC, n=N_FULL, q=128, repeat=1, mode="full", io_bufs=7,
             work_bufs=3, x2_preload=True, all_sync=True, taper=False,
             eager_out="split", xk_ring="sync"):
    """Build the per-core Bass program (SPMD: same program, per-core data).

    repeat>1 wraps the whole computation in a device-side For_i loop; used
    only for benchmarking (wall-time slope over repeat = pure HW time).
    mode: "full" | "dma" (loads only) | "nope" (no matmuls) - ablations.

    all_sync: all big loads on the single SP HWDGE ring (measured ~7%
      faster than interleaving x0/x1 on two rings).
    taper: last batch ends with q/2, q/4, q/4 tiles so the exposed
      compute tail after the final DMA byte is short.
    eager_out: evacuate batch b's PSUM during batch b+1, so the copy's
      sem-wait never stalls casts in the ACT FIFO and only the last
      batch's tiny chain remains at the end.  True = copy + out-slice
      DMA; "split" = eager ACT copies only, one full out-DMA at the end
      (the out-slice DMA would park at the SP sequencer waiting on its
      copy, delaying later loads).
    """
    from contextlib import ExitStack

    import concourse.bacc as bacc
    import concourse.mybir as mybir
    from concourse import tile
    from concourse.bass import ts  # noqa: F401

    f32 = mybir.dt.float32
    f16 = mybir.dt.float16
    nc = bacc.Bacc("TRN2", target_bir_lowering=False, debug=False)

    x0 = nc.dram_tensor("x0", [b_pc, n, 9, 2], f32, kind="ExternalInput")
    x1 = nc.dram_tensor("x1", [b_pc, n, 9, 2], f32, kind="ExternalInput")
    x2 = nc.dram_tensor("x2", [b_pc, n, 2], f32, kind="ExternalInput")
    out = nc.dram_tensor("out", [FD_S, b_pc * FD_M], f32, kind="ExternalOutput")

    nn = n // P  # voxels per partition per batch
    assert n % (P * q) == 0 and q % KGRP == 0
    n_tiles = n // (P * q)

    with ExitStack() as ctx:
        tc = ctx.enter_context(tile.TileContext(nc))
        io = ctx.enter_context(tc.tile_pool(name="io", bufs=io_bufs))
        work = ctx.enter_context(tc.tile_pool(name="work", bufs=work_bufs))
        # one PSUM bank per batch, evacuated lazily (see eager_out)
        psum = ctx.enter_context(
            tc.tile_pool(name="psum", bufs=b_pc, space="PSUM")
        )
        outp = ctx.enter_context(tc.tile_pool(name="outp", bufs=1))

        if repeat > 1:
            loop = ctx.enter_context(tc.For_i(0, repeat, 1))  # noqa: F841

        stage = outp.tile([FD_S, b_pc * FD_M], f32)
        xk_eng = nc.scalar if xk_ring == "scalar" else nc.sync

        def evacuate(b, ps):
            nc.scalar.copy(stage[:, b * FD_M : (b + 1) * FD_M], ps[:])
            if eager_out != "split":
                nc.sync.dma_start(
                    out[:, b * FD_M : (b + 1) * FD_M],
                    stage[:, b * FD_M : (b + 1) * FD_M],
                )

        def emit_tile(ps, x0v, x1v, xk_all, off, qt, start, stop):
            g0 = io.tile([P, q * 18], f32, tag="g0")
            nc.sync.dma_start(
                g0[:, : qt * 18],
                x0v[:, off : off + qt].rearrange("p qq c -> p (qq c)"),
            )
            g1 = io.tile([P, q * 18], f32, tag="g1")
            g1_eng = nc.sync if all_sync else nc.scalar
            g1_eng.dma_start(
                g1[:, : qt * 18],
                x1v[:, off : off + qt].rearrange("p qq c -> p (qq c)"),
            )
            xk = xk_all[:, off * 2 : (off + qt) * 2]

            if mode == "dma":
                # consume the loads so DCE keeps them
                nc.scalar.copy(stage[0:1, 0:18], g0[0:1, 0:18])
                nc.scalar.copy(stage[0:1, 18:36], g1[0:1, 0:18])
                nc.scalar.copy(stage[0:1, 36:38], xk[0:1, 0:2])
                return

            # ---- fp16 convert of Grf (ACT), u = x2r * Gsr (GPSIMD),
            # v = x2i * Gsr (DVE); constants fold into the host fixup.
            xkv = xk.rearrange("p (qq r) -> p qq r", r=2)
            g0v = g0[:, : qt * 18].rearrange("p (qq c) -> p qq c", c=18)
            g1h = work.tile([P, q * 18], f16, tag="g1h")
            uv = work.tile([P, 2 * q * 18], f16, tag="uv")
            uvv = uv[:, : 2 * qt * 18].rearrange(
                "p (s qq c) -> p s qq c", s=2, c=18
            )
            if mode != "nocast":
                nc.scalar.copy(g1h[:, : qt * 18], g1[:, : qt * 18])
            u_eng = nc.vector if mode == "dvboth" else nc.gpsimd
            v_eng = nc.gpsimd if mode == "gpboth" else nc.vector
            u_eng.tensor_mul(
                uvv[:, 0],
                g0v,
                xkv[:, :, 0].unsqueeze(2).broadcast_to((P, qt, 18)),
            )
            v_eng.tensor_mul(
                uvv[:, 1],
                g0v,
                xkv[:, :, 1].unsqueeze(2).broadcast_to((P, qt, 18)),
            )

            if mode == "nope":
                nc.scalar.copy(stage[0:1, 0:18], uv[0:1, 0:18])
                nc.scalar.copy(stage[0:1, 18:36], g1h[0:1, 0:18])
                return

            # ---- TensorE: per 4-voxel group, one [128,72]^T@[128,144]
            # matmul; diagonal [18,18] blocks of each half accumulate
            # sum_n Grf(x)u and sum_n Grf(x)v.
            if mode == "nocast":
                # perf ablation only: stationary from the u plane (wrong
                # math, same PE shape), ACT fully idle
                statv = uv[:, : qt * 18].rearrange(
                    "p (g wc) -> p g wc", wc=KGRP * 18
                )
            else:
                statv = g1h[:, : qt * 18].rearrange(
                    "p (g wc) -> p g wc", wc=KGRP * 18
                )
            movv = uv[:, : 2 * qt * 18].rearrange(
                "p (s g w c) -> p g s (w c)", s=2, w=KGRP, c=18
            )
            n_grp_t = qt // KGRP
            for g in range(n_grp_t):
                nc.tensor.matmul(
                    ps[:],
                    statv[:, g, :],
                    movv[:, g],
                    start=(start and g == 0),
                    stop=(stop and g == n_grp_t - 1),
                )

        pss = []
        for b in range(b_pc):
            ps = psum.tile([FD_S, FD_M], f32, tag="ps")
            pss.append(ps)
            # partition-major voxel assignment: partition p owns the
            # contiguous voxel range [p*nn, (p+1)*nn) of this batch;
            # tile t covers within-partition offsets [off, off+qt).
            # (The voxel permutation is irrelevant to the reduction.)
            x0v = x0[b].rearrange("(p nn) m r -> p nn (m r)", p=P)
            x1v = x1[b].rearrange("(p nn) m r -> p nn (m r)", p=P)
            # one contiguous [P, nn*2] f32 load per batch (8 KB per
            # partition) instead of n_tiles small per-tile loads
            xk_all = io.tile([P, nn * 2], f32, tag="xk", bufs=2)
            xk_eng.dma_start(
                xk_all[:], x2[b].rearrange("(p nn) r -> p (nn r)", p=P)
            )
            if taper and b == b_pc - 1 and q >= 4 * KGRP:
                qs_list = [q] * (n_tiles - 1) + [q // 2, q // 4, q // 4]
            else:
                qs_list = [q] * n_tiles
            off = 0
            for ti, qt in enumerate(qs_list):
                emit_tile(
                    ps, x0v, x1v, xk_all, off, qt,
                    start=(ti == 0), stop=(ti == len(qs_list) - 1),
                )
                off += qt
                # by tile 2-3 the previous batch's last matmul finished
                # long ago, so the ACT copy (and out-slice DMA) never
                # actually waits (a waiting instruction would block its
                # engine's strict FIFO, stalling casts / later loads)
                ev_ti = 2 if eager_out == "split" else 3
                full_like = mode in ("full", "dvboth", "gpboth", "nocast")
                if full_like and eager_out and b > 0 and ti == ev_ti:
                    evacuate(b - 1, pss[b - 1])
            assert off == nn

        if mode in ("full", "dvboth", "gpboth", "nocast"):
            if eager_out:
                evacuate(b_pc - 1, pss[-1])
                if eager_out == "split":
                    nc.sync.dma_start(out[:], stage[:])
            else:
                for b, ps in enumerate(pss):
                    nc.scalar.copy(
                        stage[:, b * FD_M : (b + 1) * FD_M], ps[:]
                    )
                nc.sync.dma_start(out[:], stage[:])
        else:
            nc.sync.dma_start(out[:], stage[:])

    nc.compile()
    return nc


_NC_CACHE = {}


def _get_nc():
    if "nc" not in _NC_CACHE:
        _NC_CACHE["nc"] = build_nc()
    return _NC_CACHE["nc"]


def fixup(Pm):
    """[Bt, FD_S, FD_M] grouped outer products -> [Bt, 9, 2] C entries.

    The KGRP diagonal [18,18] blocks of each 72-wide half hold partial
    sums over voxels of
      U[2*(3j+k)+tb, 2*(3i+j')+ta] = sum_v Grf_tb[v,j,k] * x2r*Gsr_ta[v,i,j']
    (and V with x2i).  kai = 10*x2r + i*KI_SCALE*x2i is linear, so the
    constants fold in here.
    """
    Bt = Pm.shape[0]
    U = np.zeros((Bt, 18, 18), np.float64)
    V = np.zeros((Bt, 18, 18), np.float64)
    for w in range(KGRP):
        U += Pm[:, 18 * w : 18 * w + 18, 18 * w : 18 * w + 18]
        V += Pm[:, 18 * w : 18 * w + 18, FD_S + 18 * w : FD_S + 18 * w + 18]
    KI = _KI_SCALE
    ii, kk = np.mgrid[0:3, 0:3]
    cr = np.zeros((Bt, 3, 3), np.float64)
    ci = np.zeros((Bt, 3, 3), np.float64)
    for j in range(3):
        ae = 2 * (3 * j + kk)  # Grf col (real part) for (j,k)
        be = 2 * (3 * ii + j)  # Gsr col (real part) for (i,j)
        cr += (
            10.0 * U[:, ae, be]
            - KI * V[:, ae, be + 1]
            - 10.0 * U[:, ae + 1, be + 1]
            - KI * V[:, ae + 1, be]
        )
        ci += (
            10.0 * U[:, ae, be + 1]
            + KI * V[:, ae, be]
            + 10.0 * U[:, ae + 1, be]
            - KI * V[:, ae + 1, be + 1]
        )
    return np.stack(
        [cr.reshape(Bt, 9), ci.reshape(Bt, 9)], axis=-1
    ).astype(np.float32)


def run(x0, x1, x2, trace=False):
    from concourse.bass_utils import run_bass_kernel_spmd

    x0 = np.ascontiguousarray(np.asarray(x0), dtype=np.float32)
    x1 = np.ascontiguousarray(np.asarray(x1), dtype=np.float32)
    x2 = np.ascontiguousarray(np.asarray(x2), dtype=np.float32)
    assert x0.shape == (B_FULL, N_FULL, 9, 2), x0.shape

    nc = _get_nc()
    in_maps = [
        {
            "x0": x0[i * B_PC : (i + 1) * B_PC],
            "x1": x1[i * B_PC : (i + 1) * B_PC],
            "x2": x2[i * B_PC : (i + 1) * B_PC],
        }
        for i in range(N_CORES)
    ]
    res = run_bass_kernel_spmd(
        nc, in_maps, core_ids=list(range(N_CORES)), trace=trace
    )
    Pm = np.concatenate(
        [
            res.results[i]["out"].reshape(FD_S, B_PC, FD_M).transpose(1, 0, 2)
            for i in range(N_CORES)
        ],
        axis=0,
    )
    return fixup(Pm), res


def kernel(x0, x1, x2):
    out, _ = run(x0, x1, x2, trace=False)
    return out


def _make_sharded_fn(nc, n_cores=N_CORES, donate=False, repeat=1):
    """Mirror bass2jax.run_bass_via_pjrt's multi-core lowering, returning a
    reusable jitted callable plus metadata, so we can time repeated runs on
    persistent device buffers."""
    import jax
    import jax.core
    from jax.experimental.shard_map import shard_map
    from jax.sharding import Mesh, PartitionSpec

    from concourse import bass2jax, mybir

    bass2jax.install_neuronx_cc_hook()

    partition_name = (
        nc.partition_id_tensor.name if nc.partition_id_tensor else None
    )
    in_names, out_names, out_avals, zero_outs = [], [], [], []
    for alloc in nc.m.functions[0].allocations:
        if not isinstance(alloc, mybir.MemoryLocationSet):
            continue
        name = alloc.memorylocations[0].name
        if alloc.kind == "ExternalInput":
            if name != partition_name:
                in_names.append(name)
        elif alloc.kind == "ExternalOutput":
            shape = tuple(alloc.tensor_shape)
            dtype = mybir.dt.np(alloc.dtype)
            out_names.append(name)
            out_avals.append(jax.core.ShapedArray(shape, dtype))
            zero_outs.append(np.zeros(shape, dtype))
    n_params = len(in_names)
    all_in_names = list(in_names) + list(out_names)
    if partition_name is not None:
        all_in_names.append(partition_name)

    def _body(*args):
        ins = list(args[:n_params])
        prev_outs = list(args[n_params:])
        # `repeat` chained executions of the same NEFF inside one XLA
        # program: each round's outputs feed the next round's (donated-zero)
        # output operands, which defeats CSE and serializes the rounds, so
        # wall-time slope over `repeat` isolates pure on-device time.
        for _ in range(repeat):
            operands = ins + prev_outs
            if partition_name is not None:
                operands.append(bass2jax.partition_id_tensor())
            prev_outs = list(
                bass2jax._bass_exec_p.bind(
                    *operands,
                    out_avals=tuple(out_avals),
                    in_names=tuple(all_in_names),
                    out_names=tuple(out_names),
                    lowering_input_output_aliases=(),
                    sim_require_finite=True,
                    sim_require_nnan=True,
                    nc=nc,
                )
            )
        return tuple(prev_outs)

    devices = jax.devices()[:n_cores]
    mesh = Mesh(np.asarray(devices), ("core",))
    in_specs = (PartitionSpec("core"),) * (n_params + len(out_names))
    out_specs = (PartitionSpec("core"),) * len(out_names)
    donate_argnums = (
        tuple(range(n_params, n_params + len(out_names))) if donate else ()
    )
    fn = jax.jit(
        shard_map(
            _body, mesh=mesh, in_specs=in_specs, out_specs=out_specs,
            check_rep=False,
        ),
        donate_argnums=donate_argnums,
        keep_unused=True,
    )
    return fn, mesh, in_names, out_names, zero_outs


def bench(x0, x1, x2, repeats=(1, 65), calls=8, reps=5, nc=None, mode="full",
          build_kwargs=None):
    """Time the NEFF on-device via the repeat-slope method.

    Builds two XLA programs that chain R executions of the same NEFF
    back-to-back on device (device-side For_i); per-call dispatch overhead
    is identical for both, so exec_ns = (T(R2) - T(R1)) / (R2 - R1) is
    pure HW time.  The two legs are timed INTERLEAVED rep-by-rep and the
    median slope is taken: the ~5-10 ms per-call dispatch overhead drifts
    on a minutes scale, so timing the legs back-to-back within a rep is
    what keeps that drift out of the slope.
    """
    import time

    import jax
    from jax.sharding import NamedSharding, PartitionSpec

    x0 = np.ascontiguousarray(np.asarray(x0), dtype=np.float32)
    x1 = np.ascontiguousarray(np.asarray(x1), dtype=np.float32)
    x2 = np.ascontiguousarray(np.asarray(x2), dtype=np.float32)
    concat = {"x0": x0, "x1": x1, "x2": x2}

    bk = dict(build_kwargs or {})
    prepped = {}
    out = None
    for R in repeats:
        if R == 1 and mode == "full" and not bk:
            nc_r = nc if nc is not None else _get_nc()
        else:
            nc_r = build_nc(repeat=R, mode=mode, **bk)
        fn, mesh, in_names, out_names, zero_outs = _make_sharded_fn(nc_r)
        sh = NamedSharding(mesh, PartitionSpec("core"))
        args = [jax.device_put(concat[n], sh) for n in in_names]
        args += [
            jax.device_put(
                np.zeros((N_CORES * z.shape[0], *z.shape[1:]), z.dtype), sh
            )
            for z in zero_outs
        ]
        out = fn(*args)
        jax.block_until_ready(out)  # compile + warm
        prepped[R] = (fn, args)

    rs = sorted(prepped)
    slopes, per_call = [], {r: float("inf") for r in rs}
    for rep in range(reps):
        t = {}
        # alternate leg order between reps: timing one leg right after
        # the other leg's completion biases the first measurement, and a
        # fixed order turns that bias into a systematic slope error
        order = rs if rep % 2 == 0 else list(reversed(rs))
        for R in order:
            fn, args = prepped[R]
            t0 = time.perf_counter()
            for _ in range(calls):
                o = fn(*args)
            jax.block_until_ready(o)
            t[R] = (time.perf_counter() - t0) / calls
            per_call[R] = min(per_call[R], t[R])
            if R == rs[-1]:
                out = o
        slopes.append((t[rs[-1]] - t[rs[0]]) / (rs[-1] - rs[0]))
    slopes.sort()
    per_exec = slopes[len(slopes) // 2]
    return per_exec * 1e9, {r: f"{v*1e6:.0f}us" for r, v in per_call.items()}, (
        np.asarray(out[0]) if out is not None else None
    )
